# revision 2
# baseline (speedup 1.0000x reference)
"""Trainium2 Bass kernel for T5-style cross-attention, sharded over 8 NeuronCores.

Sharding: tensor-parallel over heads (16 heads -> 2 per core). Each core
computes Q/K/V projections for its 2 heads (full batch), flash-style
attention with additive position bias, and a partial output projection
against its row-slice of Wo. The host sums the 8 partial outputs
(the unshard step for a row-sharded Wo).

Kernel-internal layout is fully transposed (S^T = [k, q] tiles) so the
softmax denominator comes out of the PE via a ones-column appended to V,
and no on-chip transposes of the big attention matrix are needed. The
host pre-transposes x/encoding/bias once so every DMA is a contiguous
natural load. The position bias is added on the PE itself by an
identity-weight matmul accumulating into the scores PSUM bank. exp runs
on ScalarE over [128, 1024] PSUM groups. The two heads' score matmuls
(contraction = 64) are issued back-to-back at base partitions 0/64 so
they run concurrently in separate PE row groups.
"""

import sys

try:
    import concourse.bass as bass
except ImportError:
    sys.path.insert(0, "/opt/trn_rl_repo")
    import concourse.bass as bass

import numpy as np
import ml_dtypes
_ml_bf16 = ml_dtypes.bfloat16

import concourse.mybir as mybir
from concourse import bacc
from concourse.tile import TileContext
from concourse.bass_utils import run_bass_kernel_spmd

F32 = mybir.dt.float32
F32R = mybir.dt.float32r
BF16 = mybir.dt.bfloat16

# Problem sizes (hardcoded per spec)
B, NQ, NKV = 4, 2048, 2048
D_MODEL, N_HEADS, D_K = 1024, 16, 64
N_CORES = 8
HPC = N_HEADS // N_CORES          # heads per core = 2
DH = HPC * D_K                    # 128 partition rows of per-core head dims

PW = 1024                         # projection load width (2 x 512 matmuls)
QW = 256                          # flash q window
KT = 128                          # k tile (partition dim of S^T)
KG = 4                            # k tiles per exp group ([128, KG*QW] psum)


def build_kernel(b=B, nq=NQ, nkv=NKV, d_model=D_MODEL):
    nc = bacc.Bacc("TRN2", target_bir_lowering=False, debug=False,
                   num_devices=N_CORES)

    xT = nc.dram_tensor("xT", [b, d_model, nq], F32R, kind="ExternalInput")
    encT = nc.dram_tensor("encT", [b, d_model, nkv], F32R, kind="ExternalInput")
    biasT = nc.dram_tensor("biasT", [HPC, nkv, nq], BF16, kind="ExternalInput")
    wq = nc.dram_tensor("wq", [d_model, DH], F32R, kind="ExternalInput")
    wk = nc.dram_tensor("wk", [d_model, DH], F32R, kind="ExternalInput")
    wv = nc.dram_tensor("wv", [d_model, DH], F32R, kind="ExternalInput")
    wo = nc.dram_tensor("wo", [DH, d_model], F32R, kind="ExternalInput")
    consts = nc.dram_tensor("consts", [128, 129], F32R, kind="ExternalInput")
    identb = nc.dram_tensor("identb", [128, 128], BF16, kind="ExternalInput")
    out = nc.dram_tensor("out", [b, nq, d_model], F32, kind="ExternalOutput")

    n_m = d_model // 128          # model-dim tiles (8)
    pws = min(PW, nq, nkv)        # projection window size
    n_pw_q = nq // pws            # projection windows over q
    n_pw_k = nkv // pws           # projection windows over k
    n_qw = nq // QW               # flash q windows (8)
    n_kt = nkv // KT              # k tiles (16)
    n_kg = n_kt // KG             # exp groups (4)

    with TileContext(nc) as tc:
        with (
            tc.tile_pool(name="cst", bufs=1) as cst,
            tc.tile_pool(name="wpool", bufs=1) as wpool,
            tc.tile_pool(name="qkv", bufs=1) as qkv,
            tc.tile_pool(name="actst", bufs=6) as actst,
            tc.tile_pool(name="sbias", bufs=4) as sbias,
            tc.tile_pool(name="sattn", bufs=4) as sattn,
            tc.tile_pool(name="sctx", bufs=2 * b) as sctx,
            tc.tile_pool(name="vtstage", bufs=2) as vtstage,
            tc.tile_pool(name="sout", bufs=2) as sout,
            tc.tile_pool(name="ssmall", bufs=6) as ssmall,
            tc.tile_pool(name="psbig", bufs=2, space="PSUM") as psbig,
            tc.tile_pool(name="pssmall", bufs=4, space="PSUM") as pssmall,
        ):
            # ---- constants & weights ----
            ident = cst.tile([128, 128], F32R, tag="ident")
            nc.sync.dma_start(out=ident, in_=consts[:, 0:128])
            ones_col = cst.tile([128, 1], F32R, tag="ones")
            nc.sync.dma_start(out=ones_col, in_=consts[:, 128:129])
            ident_bf = cst.tile([128, 128], BF16, tag="identbf")
            nc.sync.dma_start(out=ident_bf, in_=identb[:, :])

            wq_sb = wpool.tile([128, n_m * DH], F32R, tag="wq")
            wk_sb = wpool.tile([128, n_m * DH], F32R, tag="wk")
            wv_sb = wpool.tile([128, n_m * DH], F32R, tag="wv")
            for m in range(n_m):
                nc.sync.dma_start(out=wq_sb[:, m * DH:(m + 1) * DH],
                                  in_=wq[m * 128:(m + 1) * 128, :])
                nc.sync.dma_start(out=wk_sb[:, m * DH:(m + 1) * DH],
                                  in_=wk[m * 128:(m + 1) * 128, :])
                nc.sync.dma_start(out=wv_sb[:, m * DH:(m + 1) * DH],
                                  in_=wv[m * 128:(m + 1) * 128, :])
            wo_sb = wpool.tile([128, d_model], F32R, tag="wo")
            nc.sync.dma_start(out=wo_sb, in_=wo[:, :])

            # ---- phase A: projections ----
            qT_sb = qkv.tile([128, b * nq], F32R, tag="qT")
            kT_sb = qkv.tile([128, b * nkv], F32R, tag="kT")
            # pair-packed Vones tiles: [h0 V(64) | ones | h1 V(64) | ones]
            vones = {}
            for bi in range(b):
                for kt in range(n_kt):
                    vones[(bi, kt)] = qkv.tile(
                        [128, 2 * (D_K + 1)], F32R, tag=f"v_{bi}_{kt}",
                        name=f"v_{bi}_{kt}")

            for bi in range(b):
                # Q^T projection, PW-wide input loads (2 x 512 matmul cols)
                for pw in range(n_pw_q):
                    q_ps = psbig.tile([128, pws], F32, tag="big")
                    for m in range(n_m):
                        xt = actst.tile([128, pws], F32R, tag="actst")
                        nc.sync.dma_start(
                            out=xt,
                            in_=xT[bi, m * 128:(m + 1) * 128,
                                   pw * pws:(pw + 1) * pws])
                        for s in range(pws // 512):
                            nc.tensor.matmul(
                                q_ps[:, s * 512:(s + 1) * 512],
                                wq_sb[:, m * DH:(m + 1) * DH],
                                xt[:, s * 512:(s + 1) * 512],
                                start=(m == 0), stop=(m == n_m - 1))
                    nc.vector.tensor_copy(
                        qT_sb[:, bi * nq + pw * pws: bi * nq + (pw + 1) * pws],
                        q_ps)
                # K^T and V^T projections
                for pw in range(n_pw_k):
                    k_ps = psbig.tile([128, pws], F32, tag="big")
                    v_ps = psbig.tile([128, pws], F32, tag="big")
                    for m in range(n_m):
                        et = actst.tile([128, pws], F32R, tag="actst")
                        nc.sync.dma_start(
                            out=et,
                            in_=encT[bi, m * 128:(m + 1) * 128,
                                     pw * pws:(pw + 1) * pws])
                        for s in range(pws // 512):
                            nc.tensor.matmul(
                                k_ps[:, s * 512:(s + 1) * 512],
                                wk_sb[:, m * DH:(m + 1) * DH],
                                et[:, s * 512:(s + 1) * 512],
                                start=(m == 0), stop=(m == n_m - 1))
                            nc.tensor.matmul(
                                v_ps[:, s * 512:(s + 1) * 512],
                                wv_sb[:, m * DH:(m + 1) * DH],
                                et[:, s * 512:(s + 1) * 512],
                                start=(m == 0), stop=(m == n_m - 1))
                    nc.vector.tensor_copy(
                        kT_sb[:, bi * nkv + pw * pws: bi * nkv + (pw + 1) * pws],
                        k_ps)
                    vt_win = vtstage.tile([128, pws], F32R, tag="vtw")
                    nc.vector.tensor_copy(vt_win, v_ps)
                    # V^T -> V tiles via PE transpose, plus the ones columns
                    for s in range(pws // KT):
                        kt = pw * (pws // KT) + s
                        vt_ps = pssmall.tile([128, 128], F32R, tag="small",
                                             name=f"vtp_{bi}_{kt}")
                        nc.tensor.transpose(
                            vt_ps, vt_win[:, s * KT:(s + 1) * KT], ident)
                        vt = vones[(bi, kt)]
                        for h in range(HPC):
                            o = h * (D_K + 1)
                            nc.vector.tensor_copy(
                                vt[:, o:o + D_K],
                                vt_ps[:, h * D_K:(h + 1) * D_K])
                            nc.vector.tensor_copy(
                                vt[:, o + D_K:o + D_K + 1], ones_col)

            # ---- phase B: flash attention + output projection ----
            def emit_wo(pend):
                pq0, pctx = pend
                for bi in range(b):
                    for qs in range(QW // 128):
                        o_ps = psbig.tile([128, d_model], F32, tag="big",
                                          name=f"ops_{pq0}_{bi}_{qs}")
                        for e in range(d_model // 512):
                            nc.tensor.matmul(
                                o_ps[:, e * 512:(e + 1) * 512],
                                pctx[bi][:, qs * 128:(qs + 1) * 128],
                                wo_sb[:, e * 512:(e + 1) * 512],
                                start=True, stop=True)
                        o_sb = sout.tile([128, d_model], F32, tag="out")
                        nc.vector.tensor_copy(o_sb, o_ps)
                        nc.sync.dma_start(
                            out=out[bi, pq0 + qs * 128:
                                    pq0 + (qs + 1) * 128, :],
                            in_=o_sb)

            pending_wo = None
            for qw in range(n_qw):
                q0 = qw * QW
                ctx_t = [sctx.tile([128, QW], F32R, tag="ctx",
                                   name=f"ctx_{qw}_{bi}")
                         for bi in range(b)]
                # u tile per batch: one PSUM bank, h0 in cols 0:QW, h1 in QW:2QW
                u_t = []
                for bi in range(b):
                    u = pssmall.tile([D_K + 1, 2 * QW], F32, tag="small",
                                     name=f"u_{qw}_{bi}")
                    # pre-zero + start=False on every accumulating matmul:
                    # overwrite-onto-zero and accumulate-onto-zero agree, so
                    # stale has_written state is harmless and no bank-level
                    # "start" clear is needed (two interleaved accumulation
                    # sequences share this bank).
                    nc.vector.memset(u, 0.0)
                    u_t.append(u)
                for kg in range(n_kg):
                    if kg == n_kg - 1 and pending_wo is not None:
                        emit_wo(pending_wo)
                        pending_wo = None
                    bias_sb = {}
                    for h in range(HPC):
                        bias_sb[h] = sbias.tile([128, KG * QW], BF16,
                                                tag="bias", name=f"bias_{h}")
                        nc.sync.dma_start(
                            out=bias_sb[h].rearrange("p (t q) -> p t q", t=KG),
                            in_=biasT[h, kg * KG * KT:(kg + 1) * KG * KT,
                                      q0:q0 + QW]
                            .rearrange("(t p) q -> p t q", p=KT))
                    for bi in range(b):
                        s_g = {}
                        for h in range(HPC):
                            s_g[h] = psbig.tile([128, KG * QW], F32,
                                                tag="big", name=f"sg_{h}")
                        for j in range(KG):
                            kt = kg * KG + j
                            # the two heads' score matmuls are adjacent and
                            # use PE row groups 0/1 vs 2/3 concurrently
                            for h in range(HPC):
                                hp = h * D_K
                                nc.tensor.matmul(
                                    s_g[h][:, j * QW:(j + 1) * QW],
                                    kT_sb[hp:hp + D_K,
                                          bi * nkv + kt * KT:
                                          bi * nkv + (kt + 1) * KT],
                                    qT_sb[hp:hp + D_K,
                                          bi * nq + q0: bi * nq + q0 + QW],
                                    start=True, stop=False)
                            for h in range(HPC):
                                nc.tensor.matmul(
                                    s_g[h][:, j * QW:(j + 1) * QW],
                                    ident_bf,
                                    bias_sb[h][:, j * QW:(j + 1) * QW],
                                    start=False, stop=True,
                                    skip_group_check=True)
                        for h in range(HPC):
                            attn = sattn.tile([128, KG * QW], F32R,
                                              tag="attn", name=f"attn_{h}")
                            nc.scalar.activation(
                                attn, s_g[h], mybir.ActivationFunctionType.Exp)
                            for j in range(KG):
                                kt = kg * KG + j
                                o = h * (D_K + 1)
                                nc.tensor.matmul(
                                    u_t[bi][:, h * QW:(h + 1) * QW],
                                    vones[(bi, kt)][:, o:o + D_K + 1],
                                    attn[:, j * QW:(j + 1) * QW],
                                    start=False, stop=(kt == n_kt - 1),
                                    skip_group_check=True)
                for bi in range(b):
                    for h in range(HPC):
                        hp = h * D_K
                        usrc = u_t[bi][:, h * QW:(h + 1) * QW]
                        recip = ssmall.tile([1, QW], F32, tag="recip",
                                            name=f"recip_{h}")
                        nc.vector.reciprocal(recip, usrc[D_K:D_K + 1, :])
                        rb = ssmall.tile([D_K, QW], F32, tag="rb",
                                         name=f"rb_{h}")
                        nc.gpsimd.partition_broadcast(rb, recip)
                        with nc.allow_low_precision(reason="fp32r ctx for PE"):
                            nc.vector.tensor_mul(
                                ctx_t[bi][hp:hp + D_K, :],
                                usrc[0:D_K, :], rb)
                pending_wo = (q0, ctx_t)
            emit_wo(pending_wo)
    nc.compile()
    return nc


_NC_CACHE = {}


def _get_nc():
    if "nc" not in _NC_CACHE:
        _NC_CACHE["nc"] = build_kernel()
    return _NC_CACHE["nc"]


def make_in_maps(x, encoding, position_bias, Wq, Wk, Wv, Wo):
    x = np.asarray(x, np.float32)
    encoding = np.asarray(encoding, np.float32)
    position_bias = np.asarray(position_bias, np.float32)
    Wq = np.asarray(Wq, np.float32)
    Wk = np.asarray(Wk, np.float32)
    Wv = np.asarray(Wv, np.float32)
    Wo = np.asarray(Wo, np.float32)

    xT = np.ascontiguousarray(x.transpose(0, 2, 1))
    encT = np.ascontiguousarray(encoding.transpose(0, 2, 1))
    consts = np.concatenate(
        [np.eye(128, dtype=np.float32), np.ones((128, 1), np.float32)], axis=1)
    consts = np.ascontiguousarray(consts)

    in_maps = []
    for c in range(N_CORES):
        h0 = c * HPC
        in_maps.append({
            "xT": xT,
            "encT": encT,
            "biasT": np.ascontiguousarray(
                position_bias[0, h0:h0 + HPC].transpose(0, 2, 1)
                .astype(np.dtype("bfloat16")
                        if hasattr(np, "bfloat16") else _ml_bf16)),
            "wq": np.ascontiguousarray(Wq[:, h0 * D_K:(h0 + HPC) * D_K]),
            "wk": np.ascontiguousarray(Wk[:, h0 * D_K:(h0 + HPC) * D_K]),
            "wv": np.ascontiguousarray(Wv[:, h0 * D_K:(h0 + HPC) * D_K]),
            "wo": np.ascontiguousarray(Wo[h0 * D_K:(h0 + HPC) * D_K, :]),
            "consts": consts,
            "identb": np.eye(128, dtype=_ml_bf16),
        })
    return in_maps


def kernel(x, encoding, position_bias, Wq, Wk, Wv, Wo):
    in_maps = make_in_maps(x, encoding, position_bias, Wq, Wk, Wv, Wo)
    nc = _get_nc()
    res = run_bass_kernel_spmd(nc, in_maps, list(range(N_CORES)))
    acc = res.results[0]["out"].astype(np.float32)
    for c in range(1, N_CORES):
        acc = acc + res.results[c]["out"]
    return acc



# revision 40
# speedup vs baseline: 1.5755x; 1.5755x over previous
"""Trainium2 Bass kernel for T5-style cross-attention, sharded over 8 NeuronCores.

Sharding: tensor-parallel over heads (16 heads -> 2 per core). Each core
computes Q/K/V projections for its 2 heads (full batch), flash-style
attention with additive position bias, and a partial output projection
against its row-slice of Wo. The host sums the 8 partial outputs
(the unshard step for a row-sharded Wo).

v2 design (vs the f32r baseline):
- All HBM-resident activations/weights are fp16 (halves input DMA);
  position bias is fp16 pre-arranged on the host so every bias DMA is a
  dense [128, 1024] block per (head, q-window, k-group).
- Scores S^T = [k, q] tiles accumulate in PSUM f32; the additive bias
  lands via DVE tensor_add (PSUM += SBUF) for most tiles and via an
  fp16 identity-matmul on the PE for a small slice, balancing PE vs DVE.
- exp runs on ScalarE (PSUM f32 -> SBUF bf16). The softmax denominator
  comes out of the PE via a ones-column appended to V (bf16).
- V^T -> V transposes run on the DMA XBAR (dma_start_transpose), freeing
  PE/PSUM; projection PSUM->SBUF copies run on ScalarE (Copy activation),
  freeing DVE.
- Projections for batch bi+1 are issued interleaved with attention of
  earlier batches so DMA/PE/Act pipelines from ~30us in.
- Partial outputs are written fp16; the host sums the 8 partials in f32.
"""

import sys

try:
    import concourse.bass as bass
except ImportError:
    sys.path.insert(0, "/opt/trn_rl_repo")
    import concourse.bass as bass

import numpy as np
import ml_dtypes

_bf16 = ml_dtypes.bfloat16

import concourse.mybir as mybir
from concourse import bacc
from concourse.tile import TileContext
from concourse.bass_utils import run_bass_kernel_spmd

F32 = mybir.dt.float32
F16 = mybir.dt.float16
BF16 = mybir.dt.bfloat16

# Problem sizes (hardcoded per spec)
B, NQ, NKV = 4, 2048, 2048
D_MODEL, N_HEADS, D_K = 1024, 16, 64
N_CORES = 8
HPC = N_HEADS // N_CORES          # heads per core = 2
DH = HPC * D_K                    # 128 partition rows of per-core head dims

QW = 256                          # flash q window
KT = 128                          # k tile (partition dim of S^T)
KG = 4                            # k tiles per exp group ([128, KG*QW] psum)
N_QW = NQ // QW                   # 8 q windows
N_KG = NKV // (KG * KT)           # 4 k groups
N_M = D_MODEL // 128              # 8 contraction steps for projections


def build_kernel(b=B, nq=NQ, nkv=NKV, d_model=D_MODEL, debug_taps=False):
    nc = bacc.Bacc("TRN2", target_bir_lowering=False, debug=False,
                   num_devices=N_CORES)

    xT = nc.dram_tensor("xT", [b, d_model, nq], F16, kind="ExternalInput")
    encT = nc.dram_tensor("encT", [b, d_model, nkv], F16, kind="ExternalInput")
    biasT = nc.dram_tensor("biasT", [HPC, N_QW, KT, N_KG * KG * QW], F16,
                           kind="ExternalInput")
    # weights host-packed: [128, m*DH] (partition-major, m-chunks side by side)
    wq = nc.dram_tensor("wq", [128, N_M * DH], F16, kind="ExternalInput")
    wk = nc.dram_tensor("wk", [128, N_M * DH], F16, kind="ExternalInput")
    wv = nc.dram_tensor("wv", [128, N_M * DH], F16, kind="ExternalInput")
    wo = nc.dram_tensor("wo", [DH, d_model], F16, kind="ExternalInput")
    identh = nc.dram_tensor("identh", [128, 128], F16, kind="ExternalInput")
    out = nc.dram_tensor("out", [b, nq, d_model], F16, kind="ExternalOutput")
    if debug_taps:
        qT_d = nc.dram_tensor("qT_d", [128, b * nq], F16,
                              kind="ExternalOutput")
        kT_d = nc.dram_tensor("kT_d", [128, b * nkv], F16,
                              kind="ExternalOutput")
        vones_d = nc.dram_tensor("vones_d", [b, 128, 16 * 160], BF16,
                                 kind="ExternalOutput")
        sg_d = nc.dram_tensor("sg_d", [2, 128, KG * QW], F32,
                              kind="ExternalOutput")
        attn_d = nc.dram_tensor("attn_d", [2, 128, KG * QW], BF16,
                                kind="ExternalOutput")
        u_d = nc.dram_tensor("u_d", [D_K + 1, 2 * QW], F32,
                             kind="ExternalOutput")
        ctx_d = nc.dram_tensor("ctx_d", [128, QW], F16,
                               kind="ExternalOutput")
        osb_d = nc.dram_tensor("osb_d", [128, d_model], F16,
                               kind="ExternalOutput")
        rb_d = nc.dram_tensor("rb_d", [D_K, QW], F32,
                              kind="ExternalOutput")

    n_kt = nkv // KT              # 16 k tiles

    with TileContext(nc) as tc:
        with (
            tc.tile_pool(name="wpool", bufs=1) as wpool,
            tc.tile_pool(name="qkv", bufs=1) as qkv,
            tc.tile_pool(name="actst", bufs=12) as actst,
            tc.tile_pool(name="vstage", bufs=2) as vstage,
            tc.tile_pool(name="sbias", bufs=4) as sbias,
            tc.tile_pool(name="sattn", bufs=6) as sattn,
            tc.tile_pool(name="sctx", bufs=4) as sctx,
            tc.tile_pool(name="sout", bufs=3) as sout,
            tc.tile_pool(name="ssmall", bufs=6) as ssmall,
            tc.tile_pool(name="psA", bufs=3, space="PSUM") as psA,
            tc.tile_pool(name="psU", bufs=2, space="PSUM") as psU,
            tc.tile_pool(name="dbg", bufs=4) as dbgpool,
        ):
            # ---- weights & constants (host pre-packed, one DMA each) ----
            ident_sb = wpool.tile([128, 128], F16, tag="identh")
            nc.sync.dma_start(out=ident_sb, in_=identh[:, :])
            wq_sb = wpool.tile([128, N_M * DH], F16, tag="wq")
            wk_sb = wpool.tile([128, N_M * DH], F16, tag="wk")
            wv_sb = wpool.tile([128, N_M * DH], F16, tag="wv")
            nc.sync.dma_start(out=wq_sb, in_=wq[:, :])
            nc.sync.dma_start(out=wk_sb, in_=wk[:, :])
            nc.sync.dma_start(out=wv_sb, in_=wv[:, :])
            wo_sb = wpool.tile([128, d_model], F16, tag="wo")
            nc.sync.dma_start(out=wo_sb, in_=wo[:, :])

            # ---- persistent SBUF activations ----
            qT_sb = qkv.tile([128, b * nq], F16, tag="qT")
            kT_sb = qkv.tile([128, b * nkv], F16, tag="kT")
            # per batch: packed [h0 V(64)|1|pad|h1 V(64)|1|pad] per k-tile,
            # 16 tiles side by side. Head stride is 80 (not 65) because the
            # DMA XBAR transpose writes 16-element tile rows: output column
            # offsets must be 16-element aligned or the chunks land at the
            # wrong addresses (silently).
            VW = 160
            vones = {bi: qkv.tile([128, n_kt * VW], BF16, tag=f"v_{bi}",
                                  name=f"v_{bi}") for bi in range(b)}

            # ---- bias tiles (two qw windows in flight) ----
            bias_sb = {}          # (qw, h) -> [128, N_KG*KG*QW] tile

            def issue_bias(qw):
                for h in range(HPC):
                    t = sbias.tile([128, N_KG * KG * QW], F16, tag="bias",
                                   name=f"bias_{qw}_{h}")
                    nc.sync.dma_start(out=t, in_=biasT[h, qw])
                    bias_sb[(qw, h)] = t

            # ---- projections ----
            def proj_kv(bi):
                et = []
                for m in range(N_M):
                    t = actst.tile([128, nkv], F16, tag="actst")
                    nc.sync.dma_start(
                        out=t, in_=encT[bi, m * 128:(m + 1) * 128, :])
                    et.append(t)
                # ones cols of the vones tile; transposes overwrite V blocks
                nc.gpsimd.memset(vones[bi], 1.0)
                for pw in range(nkv // 1024):
                    k_ps = psA.tile([128, 1024], F32, tag="A",
                                    name=f"kps_{bi}_{pw}")
                    v_ps = psA.tile([128, 1024], F32, tag="A",
                                    name=f"vps_{bi}_{pw}")
                    for m in range(N_M):
                        for s in range(2):
                            c = pw * 1024 + s * 512
                            nc.tensor.matmul(
                                k_ps[:, s * 512:(s + 1) * 512],
                                wk_sb[:, m * DH:(m + 1) * DH],
                                et[m][:, c:c + 512],
                                start=(m == 0), stop=(m == N_M - 1))
                            nc.tensor.matmul(
                                v_ps[:, s * 512:(s + 1) * 512],
                                wv_sb[:, m * DH:(m + 1) * DH],
                                et[m][:, c:c + 512],
                                start=(m == 0), stop=(m == N_M - 1))
                    nc.scalar.activation(
                        kT_sb[:, bi * nkv + pw * 1024:
                              bi * nkv + (pw + 1) * 1024],
                        k_ps, mybir.ActivationFunctionType.Copy)
                    vt_win = vstage.tile([128, 1024], BF16, tag="vtw")
                    nc.scalar.activation(
                        vt_win, v_ps, mybir.ActivationFunctionType.Copy)
                    # V^T -> V via DMA XBAR: one 3D transpose per (pw, h)
                    # covering the 8 k-tiles of this window
                    vdst = vones[bi].rearrange("p (t c) -> p t c", c=VW)
                    for h in range(HPC):
                        nc.sync.dma_start_transpose(
                            out=vdst[:, pw * 8:(pw + 1) * 8,
                                     h * 80:h * 80 + D_K],
                            in_=vt_win[h * D_K:(h + 1) * D_K, :])

            def proj_q(bi):
                xt = []
                for m in range(N_M):
                    t = actst.tile([128, nq], F16, tag="actst")
                    nc.sync.dma_start(
                        out=t, in_=xT[bi, m * 128:(m + 1) * 128, :])
                    xt.append(t)
                for w in range(nq // 512):
                    q_ps = psU.tile([128, 512], F32, tag="U",
                                    name=f"qps_{bi}_{w}")
                    for m in range(N_M):
                        nc.tensor.matmul(
                            q_ps,
                            wq_sb[:, m * DH:(m + 1) * DH],
                            xt[m][:, w * 512:(w + 1) * 512],
                            start=(m == 0), stop=(m == N_M - 1))
                    o = bi * nq + w * 512
                    nc.scalar.activation(
                        qT_sb[:, o:o + 512], q_ps,
                        mybir.ActivationFunctionType.Copy)

            # ---- attention stages, software-pipelined at issue level ----
            # Each (qw, bi, kg) unit flows scores -> exp -> u; the u matmuls
            # for unit t are issued during unit t+1's scores so the PE's
            # in-order queue never parks on an exp dependency. Similarly the
            # normalize (DVE/Pool) and Wo stages trail by 1 and 2 units.
            attn_t = {}           # (qw,bi,kg,h) -> exp'd attn tile (sbuf)
            u_t = {}              # (qw,bi) -> psum accumulator
            ctx_t = {}            # (qw,bi) -> sbuf ctx

            def stage_scores(qw, bi, kg):
                q0 = qw * QW
                if kg == 0:
                    u = psU.tile([D_K + 1, 2 * QW], F32, tag="U",
                                 name=f"u_{qw}_{bi}")
                    nc.vector.memset(u, 0.0)
                    u_t[(qw, bi)] = u
                # Pre-fill the bias into the scores PSUM tile BEFORE the
                # QK matmuls, so no engine sits between PE and the exp:
                # - PE path: fp16 identity matmul writes bias (start=True)
                # - DVE path: tensor_copy writes bias into PSUM; the QK
                #   matmuls then run with start=False and accumulate on
                #   top. This relies on the PSUM has_written bits being
                #   set (=accumulate) from this bank's previous fully-
                #   written accumulation group — true for every psA
                #   generation (kv projections and score tiles all write
                #   every element).
                s_g = {}
                for h in range(HPC):
                    # balance the bias prefill across PE/Act/DVE (LP on the
                    # per-engine totals: PE is cheapest per tile but nearly
                    # saturated by the matmul work)
                    if h == 0 and kg == 0:
                        eng = "pe"
                    elif h == 0 and kg < 3:
                        eng = "act"
                    else:
                        eng = "dve"
                    bseg = bias_sb[(qw, h)][:, kg * KG * QW:(kg + 1) * KG * QW]
                    s_g[h] = psA.tile([128, KG * QW], F32, tag="A",
                                      name=f"sg_{qw}_{bi}_{kg}_{h}")
                    if eng == "pe":
                        # one full-bank [128,512] ident matmul per PSUM bank:
                        # start=True clears has_written for the WHOLE bank,
                        # so the prefill must cover the bank in a single
                        # matmul (narrower start=True writes would wipe the
                        # earlier ones' has_written state and the start=False
                        # score matmuls would then overwrite, dropping bias).
                        for half in range(2):
                            nc.tensor.matmul(
                                s_g[h][:, half * 512:(half + 1) * 512],
                                ident_sb,
                                bseg[:, half * 512:(half + 1) * 512],
                                start=True, stop=False)
                    elif eng == "act":
                        nc.scalar.activation(
                            s_g[h], bseg, mybir.ActivationFunctionType.Copy)
                    else:
                        nc.vector.tensor_copy(s_g[h], bseg)
                for j in range(KG):
                    kt = kg * KG + j
                    for h in range(HPC):
                        hp = h * D_K
                        nc.tensor.matmul(
                            s_g[h][:, j * QW:(j + 1) * QW],
                            kT_sb[hp:hp + D_K,
                                  bi * nkv + kt * KT:
                                  bi * nkv + (kt + 1) * KT],
                            qT_sb[hp:hp + D_K,
                                  bi * nq + q0:bi * nq + q0 + QW],
                            start=False, stop=True,
                            skip_group_check=True)
                for h in range(HPC):
                    a = sattn.tile([128, KG * QW], BF16, tag="attn",
                                   name=f"attn_{qw}_{bi}_{kg}_{h}")
                    nc.scalar.activation(
                        a, s_g[h], mybir.ActivationFunctionType.Exp)
                    attn_t[(qw, bi, kg, h)] = a
                    if debug_taps and qw == 0 and bi == 0 and kg == 0:
                        dt = dbgpool.tile([128, KG * QW], F32,
                                          tag="dbg", name=f"dbg_sg_{h}")
                        nc.vector.tensor_copy(dt, s_g[h])
                        nc.sync.dma_start(out=sg_d[h], in_=dt)
                        nc.sync.dma_start(out=attn_d[h], in_=a)

            def stage_u(qw, bi, kg):
                u = u_t[(qw, bi)]
                for h in range(HPC):
                    a = attn_t.pop((qw, bi, kg, h))
                    for j in range(KG):
                        kt = kg * KG + j
                        o = kt * VW + h * 80
                        nc.tensor.matmul(
                            u[:, h * QW:(h + 1) * QW],
                            vones[bi][:, o:o + D_K + 1],
                            a[:, j * QW:(j + 1) * QW],
                            start=False, stop=(kt == n_kt - 1),
                            skip_group_check=True)

            def stage_norm(qw, bi):
                u = u_t.pop((qw, bi))
                if debug_taps and qw == 0 and bi == 0:
                    dt = dbgpool.tile([D_K + 1, 2 * QW], F32, tag="dbg",
                                      name="dbg_u")
                    nc.vector.tensor_copy(dt, u)
                    nc.sync.dma_start(out=u_d[:, :], in_=dt)
                ctx = sctx.tile([128, QW], F16, tag="ctx",
                                name=f"ctx_{qw}_{bi}")
                for h in range(HPC):
                    hp = h * D_K
                    usrc = u[:, h * QW:(h + 1) * QW]
                    recip = ssmall.tile([1, QW], F32, tag="recip",
                                        name=f"recip_{h}")
                    nc.vector.reciprocal(recip, usrc[D_K:D_K + 1, :])
                    rb = ssmall.tile([D_K, QW], F32, tag="rb",
                                     name=f"rb_{h}")
                    nc.gpsimd.partition_broadcast(rb, recip)
                    if debug_taps and qw == 0 and bi == 0 and h == 0:
                        nc.sync.dma_start(out=rb_d[:, :], in_=rb)
                    with nc.allow_low_precision(reason="fp16 ctx for PE"):
                        nc.vector.tensor_mul(
                            ctx[hp:hp + D_K, :], usrc[0:D_K, :], rb)
                if debug_taps and qw == 0 and bi == 0:
                    nc.sync.dma_start(out=ctx_d[:, :], in_=ctx)
                ctx_t[(qw, bi)] = ctx

            def stage_wo(qw, bi):
                q0 = qw * QW
                ctx = ctx_t.pop((qw, bi))
                for qs in range(QW // 128):
                    o_sb = sout.tile([128, d_model], F16, tag="out")
                    for s in range(2):
                        o_ps = psU.tile([128, 512], F32, tag="U",
                                        name=f"ops_{qw}_{bi}_{qs}_{s}")
                        nc.tensor.matmul(
                            o_ps,
                            ctx[:, qs * 128:(qs + 1) * 128],
                            wo_sb[:, s * 512:(s + 1) * 512],
                            start=True, stop=True)
                        with nc.allow_low_precision(reason="fp16 partials"):
                            nc.vector.tensor_copy(
                                o_sb[:, s * 512:(s + 1) * 512], o_ps)
                    if debug_taps and qw == 0 and bi == 0 and qs == 0:
                        nc.sync.dma_start(out=osb_d[:, :], in_=o_sb)
                    nc.sync.dma_start(
                        out=out[bi, q0 + qs * 128:q0 + (qs + 1) * 128, :],
                        in_=o_sb)

            # pipeline driver state
            pend_u = []           # units whose scores are issued, u pending
            pend_fin = []         # [(qw, bi, ticks_left_to_norm)]
            pend_wo_q = []

            def tick(qw, bi, kg, flush=False):
                # 1) u for the previous unit
                if pend_u and (len(pend_u) > 1 or flush or True):
                    pass
                if pend_u:
                    uq, ub, ukg = pend_u.pop(0)
                    stage_u(uq, ub, ukg)
                    if ukg == N_KG - 1:
                        pend_fin.append([uq, ub, 1])
                # 2) trailing norm
                for ent in list(pend_fin):
                    ent[2] -= 1
                    if ent[2] <= 0:
                        stage_norm(ent[0], ent[1])
                        pend_fin.remove(ent)
                        pend_wo_q.append([ent[0], ent[1], 1])
                # 3) current scores + exp
                if qw is not None:
                    stage_scores(qw, bi, kg)
                    pend_u.append((qw, bi, kg))
                # 4) trailing wo
                for ent in list(pend_wo_q):
                    ent[2] -= 1
                    if ent[2] <= 0:
                        stage_wo(ent[0], ent[1])
                        pend_wo_q.remove(ent)

            # ---- issue schedule ----
            issue_bias(0)
            issue_bias(1)
            proj_kv(0); proj_q(0)
            proj_kv(1); proj_q(1)
            for kg in range(N_KG):
                tick(0, 0, kg)
            proj_kv(2); proj_q(2)
            for kg in range(N_KG):
                tick(0, 1, kg)
            proj_kv(3); proj_q(3)
            for kg in range(N_KG):
                tick(0, 2, kg)
            for kg in range(N_KG):
                tick(0, 3, kg)
            for qw in range(1, N_QW):
                if qw + 1 < N_QW:
                    issue_bias(qw + 1)
                for bi in range(b):
                    for kg in range(N_KG):
                        tick(qw, bi, kg)
            # flush the pipeline
            for _ in range(4):
                tick(None, None, None, flush=True)
            if debug_taps:
                nc.sync.dma_start(out=qT_d[:, :], in_=qT_sb)
                nc.sync.dma_start(out=kT_d[:, :], in_=kT_sb)
                for bi in range(b):
                    nc.sync.dma_start(out=vones_d[bi], in_=vones[bi])
    nc.compile()
    return nc


_NC_CACHE = {}


def _get_nc():
    if "nc" not in _NC_CACHE:
        _NC_CACHE["nc"] = build_kernel()
    return _NC_CACHE["nc"]


def make_in_maps(x, encoding, position_bias, Wq, Wk, Wv, Wo):
    x = np.asarray(x, np.float32)
    encoding = np.asarray(encoding, np.float32)
    position_bias = np.asarray(position_bias, np.float32)
    Wq = np.asarray(Wq, np.float32)
    Wk = np.asarray(Wk, np.float32)
    Wv = np.asarray(Wv, np.float32)
    Wo = np.asarray(Wo, np.float32)

    xT = np.ascontiguousarray(x.transpose(0, 2, 1)).astype(np.float16)
    encT = np.ascontiguousarray(encoding.transpose(0, 2, 1)).astype(np.float16)
    ident = np.eye(128, dtype=np.float16)

    def pack_w(W, h0):
        # [1024, 128] head-slice -> [128, 8*128] partition-major m-chunks
        sl = W[:, h0 * D_K:(h0 + HPC) * D_K].astype(np.float16)
        return np.ascontiguousarray(
            sl.reshape(N_M, 128, DH).transpose(1, 0, 2).reshape(
                128, N_M * DH))

    in_maps = []
    for c in range(N_CORES):
        h0 = c * HPC
        # bias block layout [h, qw, p(kt-within), (kg, t, qq)] fp16
        bT = np.empty((HPC, N_QW, KT, N_KG * KG * QW), np.float16)
        for h in range(HPC):
            bh = position_bias[0, h0 + h]            # [q, k] f32
            arr = bh.reshape(N_QW, QW, N_KG, KG, KT)  # qw qq kg t p
            bT[h] = arr.transpose(0, 4, 2, 3, 1).reshape(
                N_QW, KT, N_KG * KG * QW)
        in_maps.append({
            "xT": xT,
            "encT": encT,
            "biasT": np.ascontiguousarray(bT),
            "wq": pack_w(Wq, h0),
            "wk": pack_w(Wk, h0),
            "wv": pack_w(Wv, h0),
            "wo": np.ascontiguousarray(
                Wo[h0 * D_K:(h0 + HPC) * D_K, :]).astype(np.float16),
            "identh": ident,
        })
    return in_maps


def kernel(x, encoding, position_bias, Wq, Wk, Wv, Wo):
    in_maps = make_in_maps(x, encoding, position_bias, Wq, Wk, Wv, Wo)
    nc = _get_nc()
    res = run_bass_kernel_spmd(nc, in_maps, list(range(N_CORES)))
    acc = res.results[0]["out"].astype(np.float32)
    for c in range(1, N_CORES):
        acc = acc + res.results[c]["out"].astype(np.float32)
    return acc


# revision 44
# speedup vs baseline: 1.7126x; 1.0870x over previous
"""Trainium2 Bass kernel for T5-style cross-attention, sharded over 8 NeuronCores.

Sharding: tensor-parallel over heads (16 heads -> 2 per core). Each core
computes Q/K/V projections for its 2 heads (full batch), flash-style
attention with additive position bias, and a partial output projection
against its row-slice of Wo. The host sums the 8 partial outputs
(the unshard step for a row-sharded Wo).

v2 design (vs the f32r baseline):
- All HBM-resident activations/weights are fp16 (halves input DMA);
  position bias is fp16 pre-arranged on the host so every bias DMA is a
  dense [128, 1024] block per (head, q-window, k-group).
- Scores S^T = [k, q] tiles accumulate in PSUM f32; the additive bias
  lands via DVE tensor_add (PSUM += SBUF) for most tiles and via an
  fp16 identity-matmul on the PE for a small slice, balancing PE vs DVE.
- exp runs on ScalarE (PSUM f32 -> SBUF bf16). The softmax denominator
  comes out of the PE via a ones-column appended to V (bf16).
- V^T -> V transposes run on the DMA XBAR (dma_start_transpose), freeing
  PE/PSUM; projection PSUM->SBUF copies run on ScalarE (Copy activation),
  freeing DVE.
- Projections for batch bi+1 are issued interleaved with attention of
  earlier batches so DMA/PE/Act pipelines from ~30us in.
- Partial outputs are written fp16; the host sums the 8 partials in f32.
"""

import sys

try:
    import concourse.bass as bass
except ImportError:
    sys.path.insert(0, "/opt/trn_rl_repo")
    import concourse.bass as bass

import numpy as np
import ml_dtypes

_bf16 = ml_dtypes.bfloat16

import concourse.mybir as mybir
from concourse import bacc
from concourse.tile import TileContext
from concourse.bass_utils import run_bass_kernel_spmd

F32 = mybir.dt.float32
F16 = mybir.dt.float16
BF16 = mybir.dt.bfloat16

# Problem sizes (hardcoded per spec)
B, NQ, NKV = 4, 2048, 2048
D_MODEL, N_HEADS, D_K = 1024, 16, 64
N_CORES = 8
HPC = N_HEADS // N_CORES          # heads per core = 2
DH = HPC * D_K                    # 128 partition rows of per-core head dims

QW = 256                          # flash q window
KT = 128                          # k tile (partition dim of S^T)
KG = 4                            # k tiles per exp group ([128, KG*QW] psum)
N_QW = NQ // QW                   # 8 q windows
N_KG = NKV // (KG * KT)           # 4 k groups
N_M = D_MODEL // 128              # 8 contraction steps for projections


def build_kernel(b=B, nq=NQ, nkv=NKV, d_model=D_MODEL, debug_taps=False):
    nc = bacc.Bacc("TRN2", target_bir_lowering=False, debug=False,
                   num_devices=N_CORES)

    xT = nc.dram_tensor("xT", [b, d_model, nq], F16, kind="ExternalInput")
    encT = nc.dram_tensor("encT", [b, d_model, nkv], F16, kind="ExternalInput")
    biasT = nc.dram_tensor("biasT", [HPC, N_QW, KT, N_KG * KG * QW], F16,
                           kind="ExternalInput")
    # weights host-packed: [128, m*DH] (partition-major, m-chunks side by side)
    wq = nc.dram_tensor("wq", [128, N_M * DH], F16, kind="ExternalInput")
    wk = nc.dram_tensor("wk", [128, N_M * DH], F16, kind="ExternalInput")
    wv = nc.dram_tensor("wv", [128, N_M * DH], F16, kind="ExternalInput")
    wo = nc.dram_tensor("wo", [DH, d_model], F16, kind="ExternalInput")
    identh = nc.dram_tensor("identh", [128, 128], F16, kind="ExternalInput")
    out = nc.dram_tensor("out", [b, nq, d_model], F16, kind="ExternalOutput")
    if debug_taps:
        qT_d = nc.dram_tensor("qT_d", [128, b * nq], F16,
                              kind="ExternalOutput")
        kT_d = nc.dram_tensor("kT_d", [128, b * nkv], F16,
                              kind="ExternalOutput")
        vones_d = nc.dram_tensor("vones_d", [b, 128, 16 * 160], BF16,
                                 kind="ExternalOutput")
        sg_d = nc.dram_tensor("sg_d", [2, 128, KG * QW], F32,
                              kind="ExternalOutput")
        attn_d = nc.dram_tensor("attn_d", [2, 128, KG * QW], BF16,
                                kind="ExternalOutput")
        u_d = nc.dram_tensor("u_d", [D_K + 1, 2 * QW], F32,
                             kind="ExternalOutput")
        ctx_d = nc.dram_tensor("ctx_d", [128, QW], F16,
                               kind="ExternalOutput")
        osb_d = nc.dram_tensor("osb_d", [128, d_model], F16,
                               kind="ExternalOutput")
        rb_d = nc.dram_tensor("rb_d", [D_K, QW], F32,
                              kind="ExternalOutput")

    n_kt = nkv // KT              # 16 k tiles

    with TileContext(nc) as tc:
        with (
            tc.tile_pool(name="wpool", bufs=1) as wpool,
            tc.tile_pool(name="qkv", bufs=1) as qkv,
            tc.tile_pool(name="actst", bufs=12) as actst,
            tc.tile_pool(name="vstage", bufs=2) as vstage,
            tc.tile_pool(name="sbias", bufs=4) as sbias,
            tc.tile_pool(name="sattn", bufs=6) as sattn,
            tc.tile_pool(name="sctx", bufs=4) as sctx,
            tc.tile_pool(name="sout", bufs=3) as sout,
            tc.tile_pool(name="ssmall", bufs=6) as ssmall,
            tc.tile_pool(name="psA", bufs=3, space="PSUM") as psA,
            tc.tile_pool(name="psU", bufs=2, space="PSUM") as psU,
            tc.tile_pool(name="dbg", bufs=4) as dbgpool,
        ):
            # ---- weights & constants (host pre-packed, one DMA each) ----
            ident_sb = wpool.tile([128, 128], F16, tag="identh")
            nc.sync.dma_start(out=ident_sb, in_=identh[:, :])
            wq_sb = wpool.tile([128, N_M * DH], F16, tag="wq")
            wk_sb = wpool.tile([128, N_M * DH], F16, tag="wk")
            wv_sb = wpool.tile([128, N_M * DH], F16, tag="wv")
            nc.sync.dma_start(out=wq_sb, in_=wq[:, :])
            nc.sync.dma_start(out=wk_sb, in_=wk[:, :])
            nc.sync.dma_start(out=wv_sb, in_=wv[:, :])
            wo_sb = wpool.tile([128, d_model], F16, tag="wo")
            nc.sync.dma_start(out=wo_sb, in_=wo[:, :])

            # ---- persistent SBUF activations ----
            qT_sb = qkv.tile([128, b * nq], F16, tag="qT")
            kT_sb = qkv.tile([128, b * nkv], F16, tag="kT")
            # per batch: packed [h0 V(64)|1|pad|h1 V(64)|1|pad] per k-tile,
            # 16 tiles side by side. Head stride is 80 (not 65) because the
            # DMA XBAR transpose writes 16-element tile rows: output column
            # offsets must be 16-element aligned or the chunks land at the
            # wrong addresses (silently).
            VW = 160
            vones = {bi: qkv.tile([128, n_kt * VW], BF16, tag=f"v_{bi}",
                                  name=f"v_{bi}") for bi in range(b)}

            # ---- bias tiles (two qw windows in flight) ----
            bias_sb = {}          # (qw, h) -> [128, N_KG*KG*QW] tile

            def issue_bias(qw):
                for h in range(HPC):
                    t = sbias.tile([128, N_KG * KG * QW], F16, tag="bias",
                                   name=f"bias_{qw}_{h}")
                    nc.sync.dma_start(out=t, in_=biasT[h, qw])
                    bias_sb[(qw, h)] = t

            # ---- projections ----
            def proj_kv(bi):
                et = []
                for m in range(N_M):
                    t = actst.tile([128, nkv], F16, tag="actst")
                    nc.sync.dma_start(
                        out=t, in_=encT[bi, m * 128:(m + 1) * 128, :])
                    et.append(t)
                # ones cols of the vones tile; transposes overwrite V blocks
                nc.gpsimd.memset(vones[bi], 1.0)
                for pw in range(nkv // 1024):
                    k_ps = psA.tile([128, 1024], F32, tag="A",
                                    name=f"kps_{bi}_{pw}")
                    v_ps = psA.tile([128, 1024], F32, tag="A",
                                    name=f"vps_{bi}_{pw}")
                    for m in range(N_M):
                        for s in range(2):
                            c = pw * 1024 + s * 512
                            nc.tensor.matmul(
                                k_ps[:, s * 512:(s + 1) * 512],
                                wk_sb[:, m * DH:(m + 1) * DH],
                                et[m][:, c:c + 512],
                                start=(m == 0), stop=(m == N_M - 1))
                            nc.tensor.matmul(
                                v_ps[:, s * 512:(s + 1) * 512],
                                wv_sb[:, m * DH:(m + 1) * DH],
                                et[m][:, c:c + 512],
                                start=(m == 0), stop=(m == N_M - 1))
                    nc.scalar.activation(
                        kT_sb[:, bi * nkv + pw * 1024:
                              bi * nkv + (pw + 1) * 1024],
                        k_ps, mybir.ActivationFunctionType.Copy)
                    vt_win = vstage.tile([128, 1024], BF16, tag="vtw")
                    nc.scalar.activation(
                        vt_win, v_ps, mybir.ActivationFunctionType.Copy)
                    # V^T -> V via DMA XBAR: one 3D transpose per (pw, h)
                    # covering the 8 k-tiles of this window
                    vdst = vones[bi].rearrange("p (t c) -> p t c", c=VW)
                    for h in range(HPC):
                        nc.sync.dma_start_transpose(
                            out=vdst[:, pw * 8:(pw + 1) * 8,
                                     h * 80:h * 80 + D_K],
                            in_=vt_win[h * D_K:(h + 1) * D_K, :])

            def proj_q(bi):
                xt = []
                for m in range(N_M):
                    t = actst.tile([128, nq], F16, tag="actst")
                    nc.sync.dma_start(
                        out=t, in_=xT[bi, m * 128:(m + 1) * 128, :])
                    xt.append(t)
                for w in range(nq // 512):
                    q_ps = psU.tile([128, 512], F32, tag="U",
                                    name=f"qps_{bi}_{w}")
                    for m in range(N_M):
                        nc.tensor.matmul(
                            q_ps,
                            wq_sb[:, m * DH:(m + 1) * DH],
                            xt[m][:, w * 512:(w + 1) * 512],
                            start=(m == 0), stop=(m == N_M - 1))
                    o = bi * nq + w * 512
                    nc.scalar.activation(
                        qT_sb[:, o:o + 512], q_ps,
                        mybir.ActivationFunctionType.Copy)

            # ---- attention stages, software-pipelined at issue level ----
            # Each (qw, bi, kg) unit flows scores -> exp -> u; the u matmuls
            # for unit t are issued during unit t+1's scores so the PE's
            # in-order queue never parks on an exp dependency. Similarly the
            # normalize (DVE/Pool) and Wo stages trail by 1 and 2 units.
            attn_t = {}           # (qw,bi,kg,h) -> exp'd attn tile (sbuf)
            u_t = {}              # (qw,bi) -> psum accumulator
            ctx_t = {}            # (qw,bi) -> sbuf ctx

            def stage_scores(qw, bi, kg):
                q0 = qw * QW
                if kg == 0:
                    u = psU.tile([D_K + 1, 2 * QW], F32, tag="U",
                                 name=f"u_{qw}_{bi}")
                    nc.vector.memset(u, 0.0)
                    u_t[(qw, bi)] = u
                # Pre-fill the bias into the scores PSUM tile BEFORE the
                # QK matmuls, so no engine sits between PE and the exp:
                # - PE path: fp16 identity matmul writes bias (start=True)
                # - DVE path: tensor_copy writes bias into PSUM; the QK
                #   matmuls then run with start=False and accumulate on
                #   top. This relies on the PSUM has_written bits being
                #   set (=accumulate) from this bank's previous fully-
                #   written accumulation group — true for every psA
                #   generation (kv projections and score tiles all write
                #   every element).
                s_g = {}
                for h in range(HPC):
                    # balance the bias prefill across PE/Act/DVE (LP on the
                    # per-engine totals: PE is cheapest per tile but nearly
                    # saturated by the matmul work)
                    if h == 0 and kg == 0:
                        eng = "pe"
                    elif h == 0 and kg < 3:
                        eng = "act"
                    else:
                        eng = "dve"
                    bseg = bias_sb[(qw, h)][:, kg * KG * QW:(kg + 1) * KG * QW]
                    s_g[h] = psA.tile([128, KG * QW], F32, tag="A",
                                      name=f"sg_{qw}_{bi}_{kg}_{h}")
                    if eng == "pe":
                        # one full-bank [128,512] ident matmul per PSUM bank:
                        # start=True clears has_written for the WHOLE bank,
                        # so the prefill must cover the bank in a single
                        # matmul (narrower start=True writes would wipe the
                        # earlier ones' has_written state and the start=False
                        # score matmuls would then overwrite, dropping bias).
                        for half in range(2):
                            nc.tensor.matmul(
                                s_g[h][:, half * 512:(half + 1) * 512],
                                ident_sb,
                                bseg[:, half * 512:(half + 1) * 512],
                                start=True, stop=False)
                    elif eng == "act":
                        nc.scalar.activation(
                            s_g[h], bseg, mybir.ActivationFunctionType.Copy)
                    else:
                        nc.vector.tensor_copy(s_g[h], bseg)
                for j in range(KG):
                    kt = kg * KG + j
                    for h in range(HPC):
                        hp = h * D_K
                        nc.tensor.matmul(
                            s_g[h][:, j * QW:(j + 1) * QW],
                            kT_sb[hp:hp + D_K,
                                  bi * nkv + kt * KT:
                                  bi * nkv + (kt + 1) * KT],
                            qT_sb[hp:hp + D_K,
                                  bi * nq + q0:bi * nq + q0 + QW],
                            start=False, stop=True,
                            skip_group_check=True)
                for h in range(HPC):
                    a = sattn.tile([128, KG * QW], BF16, tag="attn",
                                   name=f"attn_{qw}_{bi}_{kg}_{h}")
                    nc.scalar.activation(
                        a, s_g[h], mybir.ActivationFunctionType.Exp)
                    attn_t[(qw, bi, kg, h)] = a
                    if debug_taps and qw == 0 and bi == 0 and kg == 0:
                        dt = dbgpool.tile([128, KG * QW], F32,
                                          tag="dbg", name=f"dbg_sg_{h}")
                        nc.vector.tensor_copy(dt, s_g[h])
                        nc.sync.dma_start(out=sg_d[h], in_=dt)
                        nc.sync.dma_start(out=attn_d[h], in_=a)

            def stage_u(qw, bi, kg):
                u = u_t[(qw, bi)]
                for h in range(HPC):
                    a = attn_t.pop((qw, bi, kg, h))
                    for j in range(KG):
                        kt = kg * KG + j
                        o = kt * VW + h * 80
                        nc.tensor.matmul(
                            u[:, h * QW:(h + 1) * QW],
                            vones[bi][:, o:o + D_K + 1],
                            a[:, j * QW:(j + 1) * QW],
                            start=False, stop=(kt == n_kt - 1),
                            skip_group_check=True)

            def stage_norm(qw, bi):
                u = u_t.pop((qw, bi))
                if debug_taps and qw == 0 and bi == 0:
                    dt = dbgpool.tile([D_K + 1, 2 * QW], F32, tag="dbg",
                                      name="dbg_u")
                    nc.vector.tensor_copy(dt, u)
                    nc.sync.dma_start(out=u_d[:, :], in_=dt)
                ctx = sctx.tile([128, QW], F16, tag="ctx",
                                name=f"ctx_{qw}_{bi}")
                for h in range(HPC):
                    hp = h * D_K
                    usrc = u[:, h * QW:(h + 1) * QW]
                    recip = ssmall.tile([1, QW], F32, tag="recip",
                                        name=f"recip_{h}")
                    nc.vector.reciprocal(recip, usrc[D_K:D_K + 1, :])
                    rb = ssmall.tile([D_K, QW], F32, tag="rb",
                                     name=f"rb_{h}")
                    nc.gpsimd.partition_broadcast(rb, recip)
                    if debug_taps and qw == 0 and bi == 0 and h == 0:
                        nc.sync.dma_start(out=rb_d[:, :], in_=rb)
                    with nc.allow_low_precision(reason="fp16 ctx for PE"):
                        nc.vector.tensor_mul(
                            ctx[hp:hp + D_K, :], usrc[0:D_K, :], rb)
                if debug_taps and qw == 0 and bi == 0:
                    nc.sync.dma_start(out=ctx_d[:, :], in_=ctx)
                ctx_t[(qw, bi)] = ctx

            def stage_wo(qw, bi):
                q0 = qw * QW
                ctx = ctx_t.pop((qw, bi))
                for qs in range(QW // 128):
                    o_sb = sout.tile([128, d_model], F16, tag="out")
                    for s in range(2):
                        o_ps = psU.tile([128, 512], F32, tag="U",
                                        name=f"ops_{qw}_{bi}_{qs}_{s}")
                        nc.tensor.matmul(
                            o_ps,
                            ctx[:, qs * 128:(qs + 1) * 128],
                            wo_sb[:, s * 512:(s + 1) * 512],
                            start=True, stop=True)
                        with nc.allow_low_precision(reason="fp16 partials"):
                            nc.vector.tensor_copy(
                                o_sb[:, s * 512:(s + 1) * 512], o_ps)
                    if debug_taps and qw == 0 and bi == 0 and qs == 0:
                        nc.sync.dma_start(out=osb_d[:, :], in_=o_sb)
                    nc.sync.dma_start(
                        out=out[bi, q0 + qs * 128:q0 + (qs + 1) * 128, :],
                        in_=o_sb)

            # pipeline driver state
            pend_u = []           # units whose scores are issued, u pending
            pend_fin = []         # [(qw, bi, ticks_left_to_norm)]
            pend_wo_q = []

            def tick(qw, bi, kg, flush=False):
                # 1) u for the previous unit
                if pend_u and (len(pend_u) > 1 or flush or True):
                    pass
                if pend_u:
                    uq, ub, ukg = pend_u.pop(0)
                    stage_u(uq, ub, ukg)
                    if ukg == N_KG - 1:
                        pend_fin.append([uq, ub, 1])
                # 2) trailing norm
                for ent in list(pend_fin):
                    ent[2] -= 1
                    if ent[2] <= 0:
                        stage_norm(ent[0], ent[1])
                        pend_fin.remove(ent)
                        pend_wo_q.append([ent[0], ent[1], 1])
                # 3) current scores + exp
                if qw is not None:
                    stage_scores(qw, bi, kg)
                    pend_u.append((qw, bi, kg))
                # 4) trailing wo
                for ent in list(pend_wo_q):
                    ent[2] -= 1
                    if ent[2] <= 0:
                        stage_wo(ent[0], ent[1])
                        pend_wo_q.remove(ent)

            # ---- issue schedule ----
            issue_bias(0)
            issue_bias(1)
            proj_kv(0); proj_q(0)
            proj_kv(1); proj_q(1)
            for kg in range(N_KG):
                tick(0, 0, kg)
            proj_kv(2); proj_q(2)
            for kg in range(N_KG):
                tick(0, 1, kg)
            proj_kv(3); proj_q(3)
            for kg in range(N_KG):
                tick(0, 2, kg)
            for kg in range(N_KG):
                tick(0, 3, kg)
            for qw in range(1, N_QW):
                if qw + 1 < N_QW:
                    issue_bias(qw + 1)
                for bi in range(b):
                    for kg in range(N_KG):
                        tick(qw, bi, kg)
            # flush the pipeline
            for _ in range(4):
                tick(None, None, None, flush=True)
            if debug_taps:
                nc.sync.dma_start(out=qT_d[:, :], in_=qT_sb)
                nc.sync.dma_start(out=kT_d[:, :], in_=kT_sb)
                for bi in range(b):
                    nc.sync.dma_start(out=vones_d[bi], in_=vones[bi])
    nc.compile()
    return nc


_NC_CACHE = {}


def _get_nc():
    if "nc" not in _NC_CACHE:
        _NC_CACHE["nc"] = build_kernel()
    return _NC_CACHE["nc"]


def make_in_maps(x, encoding, position_bias, Wq, Wk, Wv, Wo):
    x = np.asarray(x, np.float32)
    encoding = np.asarray(encoding, np.float32)
    position_bias = np.asarray(position_bias, np.float32)
    Wq = np.asarray(Wq, np.float32)
    Wk = np.asarray(Wk, np.float32)
    Wv = np.asarray(Wv, np.float32)
    Wo = np.asarray(Wo, np.float32)

    xT = np.ascontiguousarray(x.transpose(0, 2, 1)).astype(np.float16)
    encT = np.ascontiguousarray(encoding.transpose(0, 2, 1)).astype(np.float16)
    ident = np.eye(128, dtype=np.float16)

    def pack_w(W, h0):
        # [1024, 128] head-slice -> [128, 8*128] partition-major m-chunks
        sl = W[:, h0 * D_K:(h0 + HPC) * D_K].astype(np.float16)
        return np.ascontiguousarray(
            sl.reshape(N_M, 128, DH).transpose(1, 0, 2).reshape(
                128, N_M * DH))

    in_maps = []
    for c in range(N_CORES):
        h0 = c * HPC
        # bias block layout [h, qw, p(kt-within), (kg, t, qq)] fp16
        bT = np.empty((HPC, N_QW, KT, N_KG * KG * QW), np.float16)
        for h in range(HPC):
            bh = position_bias[0, h0 + h]            # [q, k] f32
            arr = bh.reshape(N_QW, QW, N_KG, KG, KT)  # qw qq kg t p
            bT[h] = arr.transpose(0, 4, 2, 3, 1).reshape(
                N_QW, KT, N_KG * KG * QW)
        in_maps.append({
            "xT": xT,
            "encT": encT,
            "biasT": np.ascontiguousarray(bT),
            "wq": pack_w(Wq, h0),
            "wk": pack_w(Wk, h0),
            "wv": pack_w(Wv, h0),
            "wo": np.ascontiguousarray(
                Wo[h0 * D_K:(h0 + HPC) * D_K, :]).astype(np.float16),
            "identh": ident,
        })
    return in_maps


def kernel(x, encoding, position_bias, Wq, Wk, Wv, Wo):
    in_maps = make_in_maps(x, encoding, position_bias, Wq, Wk, Wv, Wo)
    nc = _get_nc()
    res = run_bass_kernel_spmd(nc, in_maps, list(range(N_CORES)))
    acc = res.results[0]["out"].astype(np.float32)
    for c in range(1, N_CORES):
        acc = acc + res.results[c]["out"].astype(np.float32)
    return acc
